# revision 2
# baseline (speedup 1.0000x reference)
"""Trainium2 Bass kernel for 2-layer GAT (nn_GAT_62182536511740), v2.

Strategy (slot-major message passing, 256B rows, balanced chunks):
  * Table rows are h-only (128 fp16 = 256B, the dma_gather minimum).  Per-edge
    attention inputs (es[src] per head + ln(multiplicity)) are streamed as a
    dense vt table, host-permuted into edge-slot order between launches;
    ed[dst] is a small per-block table.  w = exp(leaky(es+ed) + ln m) with
    padded slots killed via ln m = -30000.
  * Host planner: dedup edges; duplicate the highest-out-degree sources into a
    second chunk (4 x 32768 int16 budget); 2-choice per-dst chunk balancing +
    canonical profile shaping; degree/profile sort into 128-node blocks;
    snake over 8 cores; groups of G=7 blocks with uniform per-(group,chunk)
    stripe counts S so every vector op is one big strided AP op.
  * Per (group, chunk): one gather tile filled by <=1024-idx dma_gather
    sub-calls (hard HW limit), then 4 batched DVE/ACT ops build messages.
    Matmul accumulation is per-block contiguous (identity lhsT into PSUM),
    so only ~2 PSUM accumulators are live at a time.
  * 3 launches (hext; msg layer1 + h2 rows; msg layer2 + log_softmax), host
    between launches only permutes/scatters device outputs.
"""
import numpy as np

import concourse.bacc as bacc
import concourse.bass as bass
import concourse.mybir as mybir
import concourse.tile as tile
from concourse.vector_clock import ScopedClock

# ----------------------------------------------------------------------------
N_NODES = 100000
N_EDGES = 1600000
D_IN = 128
H = 4
D_HID = 32
D_OUT = 32
NEG_SLOPE = 0.2

NCORES = 8
NCHUNKS = 4
CAP = 32768        # rows per chunk (int16 index range)
ROWE = 128         # fp16 elems per table row (256 B)
G = 6              # blocks per group (PSUM: 6 accum banks + 2 finalize)
MAXSTRIPE = 8      # stripes per dma_gather sub-call (1024 idx hard limit)
PAD_LM = -30000.0  # ln-multiplicity value for padded slots -> w = 0

F32 = mybir.dt.float32
F16 = mybir.dt.float16
I16 = mybir.dt.int16
A = mybir.AluOpType

# ----------------------------------------------------------------------------
# toolchain workarounds (walrus rejects instructions with many sync waits)


def _split_waits(nc, max_waits=1):
    for bb in nc.main_func.blocks:
        insts = bb.instructions
        i = 0
        while i < len(insts):
            ins = insts[i]
            si = ins.sync_info
            if si is not None and si.on_wait and len(si.on_wait) > max_waits:
                waits = list(si.on_wait)
                keep = waits[-max_waits:]
                move = waits[: len(waits) - len(keep)]
                del si.on_wait[:]
                si.on_wait.extend(keep)
                new_nops = []
                for w in move:
                    nop = nc.engines[ins.engine].nop(nofuse=True)
                    nop_ins = nop.ins
                    emitted = nc.cur_bb.bb.instructions
                    assert emitted[-1] is nop_ins
                    emitted.pop()
                    if nop_ins.sync_info is None:
                        nop_ins.sync_info = mybir.SyncInfo(on_wait=[w], on_update=[])
                    else:
                        nop_ins.sync_info.on_wait.append(w)
                    new_nops.append(nop_ins)
                insts[i:i] = new_nops
                i += len(new_nops)
            i += 1


def _drain_and_barrier_split(self, tick_clock, wait_clock):
    nc = self.nc
    drain_inst = nc.sync.drain()
    wait_clock.add_sem_waits(
        drain_inst.ins, ScopedClock({None: tick_clock.global_clock})
    )
    si = drain_inst.ins.sync_info
    if si is not None and si.on_wait and len(si.on_wait) > 1:
        waits = list(si.on_wait)
        del si.on_wait[:]
        bb = nc.cur_bb.bb
        assert bb.instructions[-1] is drain_inst.ins
        bb.instructions.pop()
        for w in waits:
            nop = nc.sync.nop(nofuse=True)
            nsi = nop.ins.sync_info
            if nsi is None:
                nop.ins.sync_info = mybir.SyncInfo(on_wait=[w], on_update=[])
            else:
                nsi.on_wait.append(w)
        bb.instructions.append(drain_inst.ins)
    nc.all_engine_barrier()
    assert self.sems is not None
    popped = nc._tile_sem_poison_stack.pop()
    assert popped is self._sem_poison
    nc.clear_and_free_semaphores(list(self.sems.allocated().values()))
    nc.all_engine_barrier()


tile.TileContext._drain_and_barrier = _drain_and_barrier_split


# ----------------------------------------------------------------------------
# host planning (pure indexing / graph preprocessing)

def build_plan(edge, n_nodes, cap=CAP, ncores=NCORES, g_blocks=G):
    src = np.asarray(edge[0], np.int64)
    dst = np.asarray(edge[1], np.int64)

    # --- dedup (src,dst) -> multiplicity
    key = dst * n_nodes + src
    uk, counts = np.unique(key, return_counts=True)
    udst = (uk // n_nodes).astype(np.int64)
    usrc = (uk % n_nodes).astype(np.int64)
    lm_edge = np.log(counts.astype(np.float64)).astype(np.float32)
    Eu = len(uk)

    # --- node copies: duplicate highest-out-degree srcs into a 2nd chunk
    budget = NCHUNKS * cap - n_nodes
    odeg = np.bincount(usrc, minlength=n_nodes)
    ndup = min(budget, n_nodes)
    if ndup > 0:
        dup_nodes = np.argpartition(odeg, -ndup)[-ndup:]
    else:
        dup_nodes = np.array([], np.int64)
    isdup = np.zeros(n_nodes, bool)
    isdup[dup_nodes] = True

    # chunk assignment of copies, filling all chunks to equal size
    p1 = -np.ones(n_nodes, np.int64)
    p2 = -np.ones(n_nodes, np.int64)
    pairs = [(0, 1), (2, 3), (0, 2), (1, 3), (0, 3), (1, 2)]
    dn = np.sort(dup_nodes)
    for i, n in enumerate(dn):
        a, b = pairs[i % 6]
        p1[n], p2[n] = a, b
    singles = np.nonzero(~isdup)[0]
    p1[singles] = np.arange(len(singles)) % NCHUNKS
    # per-chunk sizes
    csz = np.bincount(p1[p1 >= 0], minlength=NCHUNKS) + \
        np.bincount(p2[p2 >= 0], minlength=NCHUNKS)
    assert csz.max() <= cap, csz

    # --- edge chunk choice: 2-choice balancing per dst
    ec = p1[usrc].copy()
    flex = isdup[usrc]
    alt = np.where(flex, np.where(ec == p1[usrc], p2[usrc], p1[usrc]), ec)
    eidx = np.arange(Eu)
    for it in range(6):
        cnt = np.bincount(udst * NCHUNKS + ec, minlength=n_nodes * NCHUNKS
                          ).reshape(n_nodes, NCHUNKS)
        cur_c = cnt[udst, ec]
        alt_c = cnt[udst, alt]
        move = flex & (cur_c - alt_c >= 2) & ((eidx % 2) == (it % 2))
        ec = np.where(move, alt, ec)
        alt = np.where(move, np.where(ec == p1[usrc], p2[usrc], p1[usrc]), alt)
    # shaping toward canonical profile (ceil counts on low chunk indices)
    deg = np.bincount(udst, minlength=n_nodes)
    base = deg // NCHUNKS
    rem = deg % NCHUNKS
    target = base[:, None] + (np.arange(NCHUNKS)[None, :] < rem[:, None])
    for it in range(4):
        cnt = np.bincount(udst * NCHUNKS + ec, minlength=n_nodes * NCHUNKS
                          ).reshape(n_nodes, NCHUNKS)
        over = cnt[udst, ec] > target[udst, ec]
        under = cnt[udst, alt] < target[udst, alt]
        move = flex & over & under & ((eidx % 2) == (it % 2))
        ec = np.where(move, alt, ec)
        alt = np.where(move, np.where(ec == p1[usrc], p2[usrc], p1[usrc]), alt)

    kcnt = np.bincount(udst * NCHUNKS + ec, minlength=n_nodes * NCHUNKS
                       ).reshape(n_nodes, NCHUNKS)

    # --- block formation: sort by worst-chunk count then profile; snake cores
    kmx = kcnt.max(1)
    order = np.lexsort((kcnt[:, 3], kcnt[:, 2], kcnt[:, 1], kcnt[:, 0], kmx))
    nblk_tot = (n_nodes + 127) // 128
    NB = (nblk_tot + ncores - 1) // ncores
    NBp = NB
    slots = -np.ones(NBp * 128 * ncores, np.int64)
    slots[:n_nodes] = order
    # block b (global, in degree order) -> core via snake, j = round
    node_core = -np.ones(n_nodes, np.int64)
    node_blk = -np.ones(n_nodes, np.int64)
    node_part = -np.ones(n_nodes, np.int64)
    core_nodes = -np.ones((ncores, NBp * 128), np.int64)
    bi = 0
    for j in range(NBp):
        for c_ in range(ncores):
            core = c_ if (j % 2 == 0) else (ncores - 1 - c_)
            blk = slots[bi * 128:(bi + 1) * 128]
            core_nodes[core, j * 128:(j + 1) * 128] = blk
            sel = blk >= 0
            node_core[blk[sel]] = core
            node_blk[blk[sel]] = j
            node_part[blk[sel]] = np.nonzero(sel)[0] + 0
            bi += 1
    # fix node_part: position within block
    for core in range(ncores):
        cn = core_nodes[core]
        pos = np.nonzero(cn >= 0)[0]
        node_part[cn[pos]] = pos % 128

    # --- per-(block,chunk) stripe counts, uniform over cores (SPMD schedule)
    ecore = node_core[udst]
    eblk = node_blk[udst]
    epart = node_part[udst]
    cnt4 = np.zeros((ncores, NBp, NCHUNKS, 128), np.int32)
    np.add.at(cnt4, (ecore, eblk, ec, epart), 1)
    Sjc = cnt4.max(axis=(0, 3)).astype(np.int64)          # [NBp, NCHUNKS]

    # --- variable-size groups: <=g_blocks blocks, per-chunk slots <= KCAP
    KCAP = 88
    groups = []   # (j0, nb)
    j0 = 0
    while j0 < NBp:
        nb = 1
        while nb < g_blocks and j0 + nb < NBp:
            if Sjc[j0:j0 + nb + 1].sum(0).max() > KCAP:
                break
            nb += 1
        groups.append((j0, nb))
        j0 += nb
    ngrp = len(groups)

    # slot columns: for g, for c, for j in g: Sjc[j,c] stripes (block-major)
    colbase = np.zeros((ngrp, NCHUNKS), np.int64)   # start of (g,c) segment
    blkbase = np.zeros((NBp, NCHUNKS), np.int64)    # start of block j in seg
    t = 0
    for g, (gj0, gnb) in enumerate(groups):
        for c in range(NCHUNKS):
            colbase[g, c] = t
            for j in range(gj0, gj0 + gnb):
                blkbase[j, c] = t
                t += Sjc[j, c]
    TOT_SLOTS = int(t)

    # --- table positions within chunks
    pos1 = -np.ones(n_nodes, np.int64)
    pos2 = -np.ones(n_nodes, np.int64)
    nxt = np.zeros(NCHUNKS, np.int64)
    # order rows within chunk by node id (any fixed order)
    for c in range(NCHUNKS):
        sel1 = np.nonzero(p1 == c)[0]
        pos1[sel1] = np.arange(len(sel1))
        sel2 = np.nonzero(p2 == c)[0]
        pos2[sel2] = len(sel1) + np.arange(len(sel2))
        nxt[c] = len(sel1) + len(sel2)
    assert nxt.max() <= cap
    chunk_base = np.arange(NCHUNKS) * cap
    TABROWS = NCHUNKS * cap

    # global table row of each copy
    gpos1 = np.where(p1 >= 0, chunk_base[np.clip(p1, 0, 3)] + pos1, -1)
    gpos2 = np.where(p2 >= 0, chunk_base[np.clip(p2, 0, 3)] + pos2, -1)

    # --- per-edge slot index within (dst, chunk)
    gk = udst * NCHUNKS + ec
    eorder = np.lexsort((gk,))
    gs = gk[eorder]
    grp_start = np.r_[True, gs[1:] != gs[:-1]]
    idx_in_grp = np.arange(Eu) - np.maximum.accumulate(
        np.where(grp_start, np.arange(Eu), 0))
    eslot = np.empty(Eu, np.int64)
    eslot[eorder] = idx_in_grp
    assert (eslot < Sjc[eblk, ec]).all()

    # per-edge column in the slot-major stream
    ecol = blkbase[eblk, ec] + eslot

    # per-edge int16 gather index (position of chosen src copy in its chunk)
    use1 = ec == p1[usrc]
    epos = np.where(use1, pos1[usrc], pos2[usrc])
    assert (epos >= 0).all() and (epos < cap).all()

    # --- per-core tables
    idx_cols = TOT_SLOTS * 8
    idx_tab = np.zeros((ncores, 128, idx_cols), np.int16)
    lm_tab = np.full((ncores, 128, TOT_SLOTS), PAD_LM, np.float32)
    esrc_col = -np.ones((ncores, 128, TOT_SLOTS), np.int64)

    # flat positions: within sub-call wraps. First build full flat idx arrays
    # flat position k of slot (col, p) inside its (g,c) segment:
    #   seg-local stripe t = col - colbase[g,c]; flat k = t*128 + p
    # sub-calls split stripes into <=MAXSTRIPE pieces; idx stream column for
    # (t, p): the wrap of each sub-call covers ns*128 idxs -> ns*8 cols.
    # Since wrap(reshape(ns*8,16).T) is position-preserving per sub-call and
    # sub-call boundaries are stripe-multiples, the global column layout is
    # just the concatenation; compute it directly below.
    flatidx = np.zeros((ncores, TOT_SLOTS, 128), np.int16)  # [stripe, part]
    for core in range(ncores):
        sel = np.nonzero(ecore == core)[0]
        flatidx[core][ecol[sel], epart[sel]] = epos[sel].astype(np.int16)
        lm_tab[core, epart[sel], ecol[sel]] = lm_edge[sel]
        esrc_col[core, epart[sel], ecol[sel]] = usrc[sel]

    # segment length of (g,c) in slots
    segK = np.zeros((ngrp, NCHUNKS), np.int64)
    for g, (gj0, gnb) in enumerate(groups):
        for c in range(NCHUNKS):
            segK[g, c] = int(Sjc[gj0:gj0 + gnb, c].sum())

    # wrap into idx_tab per sub-call
    call_sched = []   # (g, c, stripe_off_in_gc, ns, iw_off)
    iw = 0
    for g in range(ngrp):
        for c in range(NCHUNKS):
            K = int(segK[g, c])
            sc = 0
            while sc < K:
                ns = min(MAXSTRIPE, K - sc)
                call_sched.append((g, c, sc, ns, iw))
                iw += ns * 8
                sc += ns
    assert iw == idx_cols
    for core in range(ncores):
        for (g, c, sc, ns, iwo) in call_sched:
            t0 = colbase[g, c] + sc
            flat = flatidx[core, t0:t0 + ns].reshape(ns * 128)
            wrap = flat.reshape(ns * 8, 16).T          # [16, ns*8]
            idx_tab[core, :, iwo:iwo + ns * 8] = np.tile(wrap, (8, 1))

    return dict(
        n_nodes=n_nodes, NB=NBp, ngrp=ngrp, groups=groups,
        Sjc=Sjc, segK=segK, colbase=colbase, blkbase=blkbase,
        TOT_SLOTS=TOT_SLOTS,
        idx_cols=idx_cols, call_sched=call_sched,
        idx_tab=idx_tab, lm_tab=lm_tab, esrc_col=esrc_col,
        core_nodes=core_nodes, node_core=node_core, node_blk=node_blk,
        node_part=node_part, gpos1=gpos1, gpos2=gpos2, TABROWS=TABROWS,
        chunk_base=chunk_base, cap=cap,
    )


# ----------------------------------------------------------------------------
# bass builders

def build_hext(seg_len, repeat=1):
    """Launch 1: h rows + es/ed for seg_len nodes.
    xT [128, seg_len] f16, We [128, 136] f16 ->
    hx [seg_len, 128] f16, ee [seg_len, 8] f16."""
    nc = bacc.Bacc("TRN2", num_swdge_queues=4)
    ntiles = (seg_len + 127) // 128
    npad = ntiles * 128
    xT = nc.dram_tensor("xT", [128, seg_len], F16, kind="ExternalInput")
    We = nc.dram_tensor("We", [128, 136], F16, kind="ExternalInput")
    hx = nc.dram_tensor("hx", [npad, ROWE], F16, kind="ExternalOutput")
    ee = nc.dram_tensor("ee", [npad, 8], F16, kind="ExternalOutput")
    with tile.TileContext(nc) as tc:
        with (
            tc.tile_pool(name="consts", bufs=1) as cpool,
            tc.tile_pool(name="big", bufs=1) as bigp,
            tc.tile_pool(name="ps", bufs=4, space="PSUM") as pp,
        ):
            we = cpool.tile([128, 136], F16)
            nc.sync.dma_start(out=we[:], in_=We[:])
            xt = bigp.tile([128, seg_len], F16)
            nc.sync.dma_start(out=xt[:], in_=xT[:])
            hbuf = bigp.tile([128, ntiles * ROWE], F16)
            ebuf = bigp.tile([128, ntiles * 8], F16)
            if ntiles * 128 > seg_len:
                nc.vector.memset(hbuf[:], 0.0)
                nc.vector.memset(ebuf[:], 0.0)
            for _rep in range(repeat):
              for t in range(ntiles):
                  nt = min(128, seg_len - t * 128)
                  ph = pp.tile([128, 136], F32)
                  nc.tensor.matmul(ph[:nt, :], lhsT=xt[:, t * 128:t * 128 + nt],
                                   rhs=we[:], start=True, stop=True)
                  nc.vector.tensor_copy(hbuf[:nt, t * ROWE:t * ROWE + ROWE],
                                        ph[:nt, 0:128])
                  nc.vector.tensor_copy(ebuf[:nt, t * 8:t * 8 + 8],
                                        ph[:nt, 128:136])
            hv = hbuf[:]
            nc.sync.dma_start(
                out=bass.AP(hx, 0, [[ROWE, 128], [128 * ROWE, ntiles],
                                    [1, ROWE]]),
                in_=bass.AP(hv.tensor, hv.offset,
                            [hv.ap[0], [ROWE, ntiles], [1, ROWE]]))
            ev = ebuf[:]
            nc.sync.dma_start(
                out=bass.AP(ee, 0, [[8, 128], [128 * 8, ntiles], [1, 8]]),
                in_=bass.AP(ev.tensor, ev.offset,
                            [ev.ap[0], [8, ntiles], [1, 8]]))
    nc.compile()
    _split_waits(nc, max_waits=1)
    return nc


def build_msg(plan, layer2, repeat=1):
    """Launch 2/3: message passing for one layer on each core.

    inputs : tab [TABROWS, ROWE] f16, idxs [128, idx_cols] i16,
             vt [128, TOT_SLOTS*5] f16 ([es0..3 | lm] per slot),
             edt [128, NB*4] f16, btile [128, 128] f32, ident [128, 128] f16,
             (layer1) w2e [128, 136] f16
    output : layer1: hx2 [NB*128, 128] f16, ee2 [NB*128, 8] f16
             layer2: outp [NB*128, 128] f32
    """
    NB, ngrp, groups = plan["NB"], plan["ngrp"], plan["groups"]
    Sjc, segK = plan["Sjc"], plan["segK"]
    colbase, blkbase = plan["colbase"], plan["blkbase"]
    TOT_SLOTS, idx_cols = plan["TOT_SLOTS"], plan["idx_cols"]
    call_sched = plan["call_sched"]
    cap, chunk_base = plan["cap"], plan["chunk_base"]

    # per-block first/last (c,s) for matmul start/stop flags
    first_cs = {}
    last_cs = {}
    for j in range(NB):
        cs = [(c, s) for c in range(NCHUNKS) for s in range(int(Sjc[j, c]))]
        if cs:
            first_cs[j] = cs[0]
            last_cs[j] = cs[-1]

    nc = bacc.Bacc("TRN2", num_swdge_queues=4)
    tab = nc.dram_tensor("tab", [plan["TABROWS"], ROWE], F16,
                         kind="ExternalInput")
    idxs = nc.dram_tensor("idxs", [128, idx_cols], I16, kind="ExternalInput")
    vt = nc.dram_tensor("vt", [128, TOT_SLOTS * 5], F16, kind="ExternalInput")
    edt = nc.dram_tensor("edt", [128, NB * 4], F16, kind="ExternalInput")
    btile = nc.dram_tensor("btile", [128, 128], F32, kind="ExternalInput")
    identt = nc.dram_tensor("ident", [128, 128], F16, kind="ExternalInput")
    if not layer2:
        w2et = nc.dram_tensor("w2e", [128, 136], F16, kind="ExternalInput")
        hx2 = nc.dram_tensor("hx2", [NB * 128, ROWE], F16,
                             kind="ExternalOutput")
        ee2 = nc.dram_tensor("ee2", [NB * 128, 8], F16, kind="ExternalOutput")
    else:
        outp = nc.dram_tensor("outp", [NB * 128, 128], F32,
                              kind="ExternalOutput")

    # group schedule: calls per group
    calls_of_group = [[] for _ in range(ngrp)]
    for (g, c, sc, ns, iwo) in call_sched:
        calls_of_group[g].append((c, sc, ns, iwo))

    qn = 0
    with tile.TileContext(nc) as tc:
        with (
            tc.tile_pool(name="consts", bufs=1) as cpool,
            tc.tile_pool(name="ip", bufs=3) as ipool,
            tc.tile_pool(name="gp", bufs=3) as gpool,
            tc.tile_pool(name="wp", bufs=2) as wpool,
            tc.tile_pool(name="mp", bufs=3) as mpool,
            tc.tile_pool(name="fin", bufs=3) as fpool,
            tc.tile_pool(name="pb", bufs=6, space="PSUM") as pbp,
            tc.tile_pool(name="px", bufs=1, space="PSUM") as pxp,
        ):
            ident = cpool.tile([128, 128], F16)
            nc.sync.dma_start(out=ident[:], in_=identt[:])
            bt = cpool.tile([128, 128], F32)
            nc.sync.dma_start(out=bt[:], in_=btile[:])
            edtt = cpool.tile([128, NB * 4], F16)
            nc.sync.dma_start(out=edtt[:], in_=edt[:])
            vtt = cpool.tile([128, TOT_SLOTS * 5], F16)
            nc.sync.dma_start(out=vtt[:], in_=vt[:])
            if not layer2:
                w2e = cpool.tile([128, 136], F16)
                nc.sync.dma_start(out=w2e[:], in_=w2et[:])

            for _rep in range(repeat):
              for g in range(ngrp):
                gj0, gnb = groups[g]
                pbs = {}
                # ---- per chunk: gather, build messages, accumulate
                for c in range(NCHUNKS):
                    K = int(segK[g, c])
                    if K == 0:
                        continue
                    # idx slice for this (g,c)
                    iw0 = None
                    tot_cols = K * 8
                    for (cc, sc, ns, iwo) in calls_of_group[g]:
                        if cc == c and sc == 0:
                            iw0 = iwo
                    assert iw0 is not None
                    it = ipool.tile([128, tot_cols], I16, tag="it")
                    nc.sync.dma_start(out=it[:],
                                      in_=idxs[:, iw0:iw0 + tot_cols])
                    gt = gpool.tile([128, K * ROWE], F16, tag="gt")
                    for (cc, sc, ns, iwo) in calls_of_group[g]:
                        if cc != c:
                            continue
                        nc.gpsimd.dma_gather(
                            gt[:, sc * ROWE:(sc + ns) * ROWE].rearrange(
                                "p (k e) -> p k e", e=ROWE),
                            tab[int(chunk_base[c]):int(chunk_base[c]) + cap, :],
                            it[:, (iwo - iw0):(iwo - iw0) + ns * 8],
                            ns * 128, ns * 128, ROWE, queue_num=qn % 4)
                        qn += 1
                    cb0 = int(colbase[g, c])
                    # w chain: t = es + ed ; leaky ; + lm ; exp -> msg[128:132]
                    wt = wpool.tile([128, K * 4], F32, tag="wt")
                    _vt = vtt[:]
                    _ed = edtt[:]
                    _wt = wt[:]
                    # per-block ed broadcast add (S varies per block)
                    for j in range(gj0, gj0 + gnb):
                        S = int(Sjc[j, c])
                        if S == 0:
                            continue
                        bb = int(blkbase[j, c]) - cb0   # slots into segment
                        es_ap = bass.AP(
                            _vt.tensor, _vt.offset + (cb0 + bb) * 5,
                            [_vt.ap[0], [5, S], [1, 4]])
                        ed_ap = bass.AP(_ed.tensor, _ed.offset + j * 4,
                                        [_ed.ap[0], [0, S], [1, 4]])
                        wt4_ap = bass.AP(_wt.tensor, _wt.offset + bb * 4,
                                         [_wt.ap[0], [4, S], [1, 4]])
                        nc.vector.tensor_tensor(out=wt4_ap, in0=es_ap,
                                                in1=ed_ap, op=A.add)
                    nc.vector.scalar_tensor_tensor(
                        out=wt[:, :K * 4], in0=wt[:, :K * 4], scalar=NEG_SLOPE,
                        in1=wt[:, :K * 4], op0=A.mult, op1=A.max)
                    lm_ap = bass.AP(_vt.tensor, _vt.offset + cb0 * 5 + 4,
                                    [_vt.ap[0], [5, K], [0, 4]])
                    wtk_ap = bass.AP(_wt.tensor, _wt.offset,
                                     [_wt.ap[0], [4, K], [1, 4]])
                    nc.vector.tensor_tensor(out=wtk_ap, in0=wtk_ap, in1=lm_ap,
                                            op=A.add)
                    msg = mpool.tile([128, K * 132], F16, tag="msg")
                    _msg = msg[:]
                    mw_ap = bass.AP(_msg.tensor, _msg.offset + 128,
                                    [_msg.ap[0], [132, K], [1, 4]])
                    nc.scalar.activation(mw_ap, wt[:, :K * 4],
                                         mybir.ActivationFunctionType.Exp)
                    mh_ap = bass.AP(_msg.tensor, _msg.offset,
                                    [_msg.ap[0], [132, K], [32, 4], [1, 32]])
                    _gt = gt[:]
                    gt_ap = bass.AP(_gt.tensor, _gt.offset,
                                    [_gt.ap[0], [128, K], [32, 4], [1, 32]])
                    mwb_ap = bass.AP(_msg.tensor, _msg.offset + 128,
                                     [_msg.ap[0], [132, K], [1, 4], [0, 32]])
                    nc.vector.tensor_tensor(out=mh_ap, in0=gt_ap, in1=mwb_ap,
                                            op=A.mult)
                    # ---- accumulate this chunk into per-block PSUM
                    for j in range(gj0, gj0 + gnb):
                        S = int(Sjc[j, c])
                        if S == 0:
                            continue
                        if j not in pbs:
                            pbs[j] = pbp.tile([128, 132], F32, tag="pb", name=f"pb{j}")
                        bb = int(blkbase[j, c]) - cb0
                        for s in range(S):
                            nc.tensor.matmul(
                                pbs[j][:], lhsT=ident[:],
                                rhs=msg[:, (bb + s) * 132:(bb + s) * 132 + 132],
                                start=((c, s) == first_cs[j]),
                                stop=((c, s) == last_cs[j]))

                # ---- finalize blocks of this group
                for j in range(gj0, gj0 + gnb):
                    if j in pbs:
                        t1 = pbs[j]
                    else:
                        zb = fpool.tile([128, 132], F32, tag="t1")
                        nc.vector.memset(zb[:], 0.0)
                        t1 = zb
                    den = fpool.tile([128, 4], F32, tag="den")
                    nc.vector.tensor_scalar_add(den[:], t1[:, 128:132], 1e-20)
                    nc.vector.reciprocal(den[:], den[:])
                    t2 = fpool.tile([128, 128], F32, tag="t2")
                    _den = den[:]
                    nc.vector.tensor_tensor(
                        out=t2[:].rearrange("p (h d) -> p h d", d=32),
                        in0=t1[:, 0:128].rearrange("p (h d) -> p h d", d=32),
                        in1=bass.AP(_den.tensor, _den.offset,
                                    [_den.ap[0], [1, 4], [0, 32]]),
                        op=A.mult)
                    nc.vector.tensor_tensor(out=t2[:], in0=t2[:], in1=bt[:],
                                            op=A.add)
                    if not layer2:
                        x2 = fpool.tile([128, 128], F16, tag="x2")
                        nc.vector.tensor_scalar_max(x2[:], t2[:], 0.0)
                        px = pxp.tile([128, 128], F16, tag="px")
                        nc.tensor.transpose(px[:], x2[:], ident[:])
                        x2t = fpool.tile([128, 128], F16, tag="x2t")
                        nc.vector.tensor_copy(x2t[:], px[:])
                        ph2 = pxp.tile([128, 136], F32, tag="ph2")
                        nc.tensor.matmul(ph2[:], lhsT=x2t[:], rhs=w2e[:],
                                         start=True, stop=True)
                        hrow = fpool.tile([128, ROWE], F16, tag="hrow")
                        nc.vector.tensor_copy(hrow[:], ph2[:, 0:128])
                        erow = fpool.tile([128, 8], F16, tag="erow")
                        nc.vector.tensor_copy(erow[:], ph2[:, 128:136])
                        nc.sync.dma_start(
                            out=hx2[j * 128:(j + 1) * 128, :], in_=hrow[:])
                        nc.sync.dma_start(
                            out=ee2[j * 128:(j + 1) * 128, :], in_=erow[:])
                    else:
                        et = fpool.tile([128, 128], F32, tag="et")
                        nc.scalar.activation(et[:], t2[:],
                                             mybir.ActivationFunctionType.Exp)
                        ssum = fpool.tile([128, 1], F32, tag="ssum")
                        nc.vector.tensor_reduce(ssum[:], et[:],
                                                axis=mybir.AxisListType.X,
                                                op=A.add)
                        nc.scalar.activation(ssum[:], ssum[:],
                                             mybir.ActivationFunctionType.Ln)
                        nc.vector.tensor_scalar_mul(ssum[:], ssum[:], -1.0)
                        orow = fpool.tile([128, 128], F32, tag="orow")
                        nc.scalar.activation(
                            orow[:], t2[:],
                            mybir.ActivationFunctionType.Identity,
                            bias=ssum[:, 0:1])
                        nc.sync.dma_start(
                            out=outp[j * 128:(j + 1) * 128, :], in_=orow[:])
    nc.compile()
    _split_waits(nc, max_waits=1)
    return nc


# ----------------------------------------------------------------------------
# host runner

def _ext_w(W, a_s, a_d):
    """[din, dout] + per-head vectors -> [din, dout+8] f16 extended weights."""
    dout = W.shape[1]
    As = np.zeros((dout, 4), np.float32)
    Ad = np.zeros((dout, 4), np.float32)
    for h in range(H):
        As[h * 32:(h + 1) * 32, h] = a_s[h]
        Ad[h * 32:(h + 1) * 32, h] = a_d[h]
    return np.concatenate([W, W @ As, W @ Ad], axis=1).astype(np.float16)


def _run(nc, in_maps):
    from concourse.bass_utils import run_bass_kernel_spmd
    return run_bass_kernel_spmd(nc, in_maps, core_ids=list(range(NCORES)),
                                trace=False).results


def run_pipeline(inputs, n_nodes, run=_run):
    edge = np.asarray(inputs["edge"])
    x = np.asarray(inputs["features"], np.float32)
    W1 = np.asarray(inputs["W1"], np.float32)
    a1s = np.asarray(inputs["a1_src"], np.float32)
    a1d = np.asarray(inputs["a1_dst"], np.float32)
    b1 = np.asarray(inputs["b1"], np.float32)
    W2 = np.asarray(inputs["W2"], np.float32)
    a2s = np.asarray(inputs["a2_src"], np.float32)
    a2d = np.asarray(inputs["a2_dst"], np.float32)
    b2 = np.asarray(inputs["b2"], np.float32)

    plan = build_plan(edge, n_nodes)
    NB = plan["NB"]
    core_nodes = plan["core_nodes"]
    gpos1, gpos2 = plan["gpos1"], plan["gpos2"]
    esrc_col = plan["esrc_col"]
    lm_tab = plan["lm_tab"]

    # ---- launch 1: hext
    seg = n_nodes // NCORES
    assert seg * NCORES == n_nodes
    nc1 = build_hext(seg)
    W1e = _ext_w(W1, a1s, a1d)
    in1 = []
    for core in range(NCORES):
        xT = np.ascontiguousarray(
            x[core * seg:(core + 1) * seg].astype(np.float16).T)
        in1.append({"xT": xT, "We": W1e})
    res1 = run(nc1, in1)
    h_rows = np.concatenate(
        [np.asarray(res1[c]["hx"])[:seg] for c in range(NCORES)], 0)
    ee_rows = np.concatenate(
        [np.asarray(res1[c]["ee"])[:seg] for c in range(NCORES)], 0)

    def make_tab(hr):
        tabf = np.zeros((plan["TABROWS"], ROWE), np.float16)
        tabf[gpos1] = hr
        sel = gpos2 >= 0
        tabf[gpos2[sel]] = hr[sel]
        return tabf

    def make_vt(es_pernode):
        """[N,4] f32/f16 -> per-core vt [128, TOT_SLOTS*5] f16."""
        T = plan["TOT_SLOTS"]
        vts = []
        for core in range(NCORES):
            v = np.zeros((128, T, 5), np.float16)
            ecs = esrc_col[core]
            valid = ecs >= 0
            v[:, :, 0:4][valid] = es_pernode[ecs[valid]]
            v[:, :, 4] = lm_tab[core].astype(np.float16)
            vts.append(v.reshape(128, T * 5))
        return vts

    def make_edt(ed_pernode):
        """[N,4] -> per-core [128, NB*4] f16."""
        edts = []
        for core in range(NCORES):
            e = np.zeros((128, NB, 4), np.float16)
            cn = core_nodes[core].reshape(NB, 128)
            for j in range(NB):
                sel = cn[j] >= 0
                e[sel, j] = ed_pernode[cn[j][sel]]
            edts.append(e.reshape(128, NB * 4))
        return edts

    tab1 = make_tab(h_rows)
    es1 = ee_rows[:, 0:4].astype(np.float16)
    ed1 = ee_rows[:, 4:8].astype(np.float16)
    vt1 = make_vt(es1)
    edt1 = make_edt(ed1)
    ident = np.eye(128, dtype=np.float16)
    b1t = np.tile(b1.reshape(1, -1), (128, 1)).astype(np.float32)
    b2t = np.tile(b2.reshape(1, -1), (128, 1)).astype(np.float32)
    W2e = _ext_w(W2, a2s, a2d)

    # ---- launch 2: layer-1 message passing + h2 rows
    nc2 = build_msg(plan, layer2=False)
    in2 = []
    for core in range(NCORES):
        in2.append({
            "tab": tab1, "idxs": plan["idx_tab"][core], "vt": vt1[core],
            "edt": edt1[core], "btile": b1t, "ident": ident, "w2e": W2e,
        })
    res2 = run(nc2, in2)

    # reassemble h2 table + es2/ed2 per node
    h2_rows = np.zeros((n_nodes, ROWE), np.float16)
    ee2_rows = np.zeros((n_nodes, 8), np.float16)
    for core in range(NCORES):
        cn = core_nodes[core]
        vm = cn >= 0
        h2_rows[cn[vm]] = np.asarray(res2[core]["hx2"])[vm]
        ee2_rows[cn[vm]] = np.asarray(res2[core]["ee2"])[vm]
    tab2 = make_tab(h2_rows)
    vt2 = make_vt(ee2_rows[:, 0:4])
    edt2 = make_edt(ee2_rows[:, 4:8])

    # ---- launch 3: layer-2 message passing + log_softmax
    nc3 = build_msg(plan, layer2=True)
    in3 = []
    for core in range(NCORES):
        in3.append({
            "tab": tab2, "idxs": plan["idx_tab"][core], "vt": vt2[core],
            "edt": edt2[core], "btile": b2t, "ident": ident,
        })
    res3 = run(nc3, in3)

    out = np.zeros((n_nodes, H * D_OUT), np.float32)
    for core in range(NCORES):
        cn = core_nodes[core]
        vm = cn >= 0
        out[cn[vm]] = np.asarray(res3[core]["outp"])[vm]
    return out


def kernel(**inputs):
    return run_pipeline(inputs, N_NODES).astype(np.float32)
